# revision 12
# baseline (speedup 1.0000x reference)
"""Trainium2 Bass kernel for nn_DefinitionProbingLSTM.

Structure (8-core SPMD, one chip):
  - hidden units H=1024 sharded 8 ways (128 per core); each core computes its
    512 gate rows per LSTM layer per step (matmuls in fp16, batch on M=64).
  - one AllGather per step carries {h0n(t), h1n(t-1)} transposed slices, so
    layer 1 lags one gather behind layer 0 and only one collective sits on the
    serial critical chain.
  - output projection vocab-sharded (4000 vocab rows/core), fp32r matmuls,
    interleaved with the recurrence every two steps (token tiles M=128).
  - h0 = initial_state @ enc_proj_W.T computed on host in fp32 (tiny).
  - embedding gather, weight layout permutation/transposition and final argmax
    run on host inside kernel().
"""

import numpy as np
import ml_dtypes

import concourse.bass as bass
import concourse.mybir as mybir
import concourse.tile as tile
from concourse import bacc
from concourse.bass_utils import run_bass_kernel_spmd
from concourse.masks import make_identity

F16 = mybir.dt.float16
F32 = mybir.dt.float32
F32R = mybir.dt.float32r
AF = mybir.ActivationFunctionType

B, T, V, E, H, EH = 64, 64, 32000, 512, 1024, 2048
S = T - 1                    # 63 recurrence steps
NC = 8                       # cores
HS = H // NC                 # 128 hidden units per core
GS = 4 * HS                  # 512 gate rows per core per layer
VS = V // NC                 # 4000 vocab rows per core
TOK = S * B                  # 4032 tokens, order t*B + b
NTILE = (TOK + 127) // 128   # 32 token tiles (last one half: 64 rows)
NCHUNK = 8                   # vocab chunks of 500 per core
CW = VS // NCHUNK            # 500


def _build_program():
    nc = bacc.Bacc("TRN2", target_bir_lowering=False, debug=False,
                   num_devices=NC)

    d_xembT = nc.dram_tensor("xembT", [E, TOK], F16, kind="ExternalInput").ap()
    d_wih0T = nc.dram_tensor("wih0T", [E, GS], F16, kind="ExternalInput").ap()
    d_whh0T = nc.dram_tensor("whh0T", [H, GS], F16, kind="ExternalInput").ap()
    d_w1T = nc.dram_tensor("w1T", [2 * H, GS], F16, kind="ExternalInput").ap()
    d_h0T = nc.dram_tensor("h0T16", [H, B], F16, kind="ExternalInput").ap()
    d_h0cT = nc.dram_tensor("h0cT16", [128, B], F16, kind="ExternalInput").ap()
    d_c0 = nc.dram_tensor("c0", [B, HS], F32, kind="ExternalInput").ap()
    d_projT = nc.dram_tensor("projT", [H, VS], F32R, kind="ExternalInput").ap()
    d_logits = nc.dram_tensor("logits", [TOK, VS], F32, kind="ExternalOutput").ap()

    with tile.TileContext(nc) as tc:
        with tc.tile_pool(name="wpool", bufs=1) as wpool, \
             tc.tile_pool(name="big", bufs=1) as bigpool, \
             tc.tile_pool(name="xs", bufs=2) as xs, \
             tc.tile_pool(name="gath", bufs=4) as gath, \
             tc.tile_pool(name="work", bufs=2) as work, \
             tc.tile_pool(name="stage", bufs=4) as stpool, \
             tc.tile_pool(name="pl", bufs=2) as plpool, \
             tc.tile_pool(name="psA", bufs=2, space="PSUM") as psA, \
             tc.tile_pool(name="psB", bufs=1, space="PSUM") as psB, \
             tc.tile_pool(name="psT", bufs=2, space="PSUM") as psT, \
             tc.tile_pool(name="psP", bufs=3, space="PSUM") as psP, \
             tc.tile_pool(name="dr", bufs=1, space="DRAM") as dr, \
             tc.tile_pool(name="drag", bufs=3, space="DRAM") as drag:

            # ---------- resident weights ----------
            s_wih0 = wpool.tile([128, E // 128, GS], F16, tag="wih0")
            nc.sync.dma_start(s_wih0[:], d_wih0T.rearrange("(k p) g -> p k g", p=128))
            s_whh0 = wpool.tile([128, H // 128, GS], F16, tag="whh0")
            nc.sync.dma_start(s_whh0[:], d_whh0T.rearrange("(k p) g -> p k g", p=128))
            s_w1 = wpool.tile([128, 2 * H // 128, GS], F16, tag="w1")
            nc.sync.dma_start(s_w1[:], d_w1T.rearrange("(k p) g -> p k g", p=128))
            s_ident = wpool.tile([128, 128], F16, tag="ident")
            make_identity(nc, s_ident[:])

            # ---------- phase 0: initial states (host-computed) ----------
            c_l0 = work.tile([B, HS], F32, tag="c0", name="c_l0_init")
            nc.sync.dma_start(c_l0[:], d_c0)
            c_l1 = work.tile([B, HS], F32, tag="c1", name="c_l1_init")
            nc.sync.dma_start(c_l1[:], d_c0)
            h0c_T = work.tile([128, B], F16, tag="h0cT")
            nc.sync.dma_start(h0c_T[:], d_h0cT)
            # G_{-1}: even slots = h0T k-tiles (odd slots unused at t=0)
            g_prev = gath.tile([128, 2 * NC, B], F16, tag="g", name="g_init")
            nc.sync.dma_start(g_prev[:, 0::2, :],
                              d_h0T.rearrange("(k p) b -> p k b", p=128))

            # ---------- phase 1: x_ih precompute -> DRAM ----------
            d_xih = dr.tile([TOK, GS], F16, tag="xih")
            for p in range(NTILE):
                rows = min(128, TOK - p * 128)
                xt = xs.tile([128, E // 128, 128], F16, tag="xemb")
                nc.gpsimd.dma_start(
                    xt[:, :, :rows],
                    d_xembT[:, p * 128:p * 128 + rows].rearrange(
                        "(k p) m -> p k m", p=128))
                psx = psP.tile([128, GS], F32, tag="pp")
                for k in range(E // 128):
                    nc.tensor.matmul(psx[:rows, :], xt[:, k, :rows],
                                     s_wih0[:, k, :],
                                     start=(k == 0), stop=(k == E // 128 - 1))
                st16 = stpool.tile([128, GS], F16, tag="st16")
                nc.vector.tensor_copy(st16[:rows, :], psx[:rows, :])
                nc.scalar.dma_start(d_xih[p * 128:p * 128 + rows, :], st16[:rows, :])

            # projection weights resident (the one big slot)
            s_proj = bigpool.tile([128, H // 128, VS], F32R, tag="big",
                                  name="s_proj")
            for k in range(H // 128):
                nc.gpsimd.dma_start(
                    s_proj[:, k, :],
                    d_projT[k * 128:(k + 1) * 128, :])

            # ---------- phase 2: recurrence + interleaved projection ----------
            h1n_T_evac = None     # DVE-evacuated h1n(t-1).T [128, B] fp16

            plT_cur = [None]

            def project_pair(gA, gB, p, lo=0, hi=NCHUNK):
                """project token tile p (steps 2p, 2p+1), vocab chunks
                [lo, hi); gA/gB hold h1 of the two steps in odd slots."""
                rows = min(128, TOK - p * 128)
                if lo == 0:
                    plT = plpool.tile([128, H // 128, 128], F32R, tag="plT")
                    nc.vector.tensor_copy(plT[:, :, 0:B], gA[:, 1::2, :])
                    if gB is not None:
                        nc.vector.tensor_copy(plT[:, :, B:2 * B], gB[:, 1::2, :])
                    plT_cur[0] = plT
                plT = plT_cur[0]
                for n in range(lo, hi):
                    psp = psP.tile([128, CW], F32, tag="pp")
                    for k in range(H // 128):
                        nc.tensor.matmul(psp[:rows, :], plT[:, k, :rows],
                                         s_proj[:, k, n * CW:(n + 1) * CW],
                                         start=(k == 0), stop=(k == H // 128 - 1))
                    st = stpool.tile([128, GS], F32, tag="st")
                    if n % 2 == 0:
                        nc.vector.tensor_copy(st[:rows, :CW], psp[:rows, :])
                    else:
                        nc.scalar.activation(st[:rows, :CW], psp[:rows, :], AF.Copy)
                    nc.gpsimd.dma_start(
                        d_logits[p * 128:p * 128 + rows, n * CW:(n + 1) * CW],
                        st[:rows, :CW])

            g_hist = {-1: g_prev}

            def nonlin(ps_or_sb, c_prev, tag_sfx, xih_sb=None):
                """gate nonlinearity chain -> (h_new fp16 [B,HS], c_new f32)."""
                if xih_sb is not None:
                    gates = work.tile([B, GS], F32, tag="gates" + tag_sfx)
                    nc.vector.tensor_add(gates[:], ps_or_sb[:], xih_sb[:])
                else:
                    gates = ps_or_sb
                sfo = work.tile([B, 3 * HS], F32, tag="sfo" + tag_sfx)
                nc.scalar.activation(sfo[:], gates[:, 0:3 * HS], AF.Sigmoid)
                tg = work.tile([B, HS], F32, tag="tg" + tag_sfx, bufs=1)
                nc.scalar.activation(tg[:], gates[:, 3 * HS:GS], AF.Tanh)
                t1 = work.tile([B, HS], F32, tag="t1" + tag_sfx, bufs=1)
                nc.vector.tensor_mul(t1[:], sfo[:, HS:2 * HS], c_prev[:])
                t2 = work.tile([B, HS], F32, tag="t2" + tag_sfx, bufs=1)
                nc.vector.tensor_mul(t2[:], sfo[:, 0:HS], tg[:])
                c_new = work.tile([B, HS], F32, tag="c" + tag_sfx)
                nc.vector.tensor_add(c_new[:], t1[:], t2[:])
                tc = work.tile([B, HS], F32, tag="tc" + tag_sfx, bufs=1)
                nc.scalar.activation(tc[:], c_new[:], AF.Tanh)
                hn = work.tile([B, HS], F16, tag="hn" + tag_sfx)
                nc.vector.tensor_mul(hn[:], sfo[:, 2 * HS:3 * HS], tc[:])
                return hn, c_new

            # pipeline: body t emits mm0(t) and layer-1 of step t-1, then
            # projection work (fills the AG(t) wait), then AG(t) carrying
            # {h0n(t), h1n(t-1)}.
            ps1_prev = None       # mm1(t-2) PSUM, consumed by nonlin1 here
            for t in range(S + 2):
                agin_sb = work.tile([128, 2, B], F16, tag="agin")
                # ---- layer 1 matmul for step t-2 first (its chain binds)
                if t >= 2 and t - 2 < S:
                    ps1 = psB.tile([B, GS], F32, tag="g1")
                    for k in range(NC):
                        nc.tensor.matmul(ps1[:], g_hist[t - 1][:, 2 * k + 1, :],
                                         s_w1[:, NC + k, :],
                                         start=(k == 0), stop=False)
                    for k in range(NC):
                        nc.tensor.matmul(ps1[:], g_hist[t - 2][:, 2 * k, :],
                                         s_w1[:, k, :],
                                         start=False, stop=(k == NC - 1))
                    ps1_prev = ps1
                # ---- layer 0 matmul for step t (xih folded in via identity)
                if t < S:
                    xih_sb = xs.tile([B, GS], F16, tag="xih_sb")
                    nc.gpsimd.dma_start(xih_sb[:], d_xih[t * B:(t + 1) * B, :])
                    ps0 = psA.tile([B, GS], F32, tag="g0")
                    nc.tensor.matmul(ps0[:], s_ident[:B, :B], xih_sb[:],
                                     start=True, stop=False)
                    for k in range(NC):
                        nc.tensor.matmul(ps0[:], g_hist[t - 1][:, 2 * k, :],
                                         s_whh0[:, k, :],
                                         start=False, stop=(k == NC - 1))
                # ---- nonlinearities (ACT/DVE run parallel to PE)
                if t < S:
                    h0n16, c_l0 = nonlin(ps0, c_l0, "0")
                    pst0 = psT.tile([128, B], F16, tag="tp")
                    nc.tensor.transpose(pst0[:], h0n16[:], s_ident[:B, :B])
                    nc.vector.tensor_copy(agin_sb[:, 0, :], pst0[:])
                if t >= 2 and t - 2 < S:
                    h1n16, c_l1 = nonlin(ps1_prev, c_l1, "1")
                    pst1 = psT.tile([128, B], F16, tag="tp")
                    nc.tensor.transpose(pst1[:], h1n16[:], s_ident[:B, :B])
                    ev = work.tile([128, B], F16, tag="h1nT")
                    nc.vector.tensor_copy(ev[:], pst1[:])
                    h1n_T_evac = ev
                # slot1 = h1n(t-2).T ; slot0 dummy after layer0 ends
                if t >= 2:
                    nc.vector.tensor_copy(agin_sb[:, 1, :], h1n_T_evac[:])
                else:
                    nc.vector.tensor_copy(agin_sb[:, 1, :], h0c_T[:])
                if t >= S:
                    nc.vector.tensor_copy(agin_sb[:, 0, :], h1n_T_evac[:])
                # ---- AllGather t: {h0n(t), h1n(t-2)}
                ag_in = drag.tile([2 * 128, B], F16, tag="agin_d")
                nc.sync.dma_start(
                    ag_in.rearrange("(j p) b -> p j b", p=128), agin_sb[:])
                ag_out = drag.tile([2 * NC * 128, B], F16, tag="agout_d",
                                   addr_space="Shared")
                nc.gpsimd.collective_compute(
                    "AllGather", mybir.AluOpType.bypass,
                    replica_groups=[list(range(NC))],
                    ins=[ag_in.opt()], outs=[ag_out.opt()],
                )
                g_t = gath.tile([128, 2 * NC, B], F16, tag="g")
                # odds (h1, feeds the binding mm1 chain) first, then evens
                nc.sync.dma_start(
                    g_t[:, 1::2, :],
                    ag_out.rearrange("(s p) b -> p s b", p=128)[:, 1::2, :])
                nc.sync.dma_start(
                    g_t[:, 0::2, :],
                    ag_out.rearrange("(s p) b -> p s b", p=128)[:, 0::2, :])
                g_hist[t] = g_t

                # ---- projection of pair p: h1(x) lives in G_{x+2}.odd
                # pair p = steps (2p, 2p+1) -> needs G_{2p+2}, G_{2p+3};
                # chunks split 4+4 across this body and the next
                if t >= 4 and t % 2 == 0:
                    project_pair(g_hist[t - 2], g_hist[t - 1], (t - 4) // 2,
                                 0, NCHUNK // 2)
                    del g_hist[t - 3]
                if t >= 5 and t % 2 == 1:
                    project_pair(None, None, (t - 5) // 2, NCHUNK // 2, NCHUNK)

            # tail: last pairs. h1(x) in G_{x+2}: pair 30 = steps 60,61
            # -> G_62, G_63 ; half tile 31 = step 62 -> G_64.
            project_pair(g_hist[S - 1], g_hist[S], NTILE - 2, NCHUNK // 2, NCHUNK)
            project_pair(g_hist[S + 1], None, NTILE - 1)

    nc.compile()
    return nc


_PROGRAM = None


def _get_program():
    global _PROGRAM
    if _PROGRAM is None:
        _PROGRAM = _build_program()
    return _PROGRAM


def kernel(input_ids, initial_state, emb, enc_proj_W, W_ih0, W_hh0, b_ih0,
           b_hh0, W_ih1, W_hh1, b_ih1, b_hh1, proj_W, proj_b):
    input_ids = np.asarray(input_ids)
    initial_state = np.asarray(initial_state, dtype=np.float32)
    emb = np.asarray(emb, dtype=np.float32)
    enc_proj_W = np.asarray(enc_proj_W, dtype=np.float32)
    proj_W = np.asarray(proj_W, dtype=np.float32)
    assert not np.any(np.asarray(b_ih0)) and not np.any(np.asarray(b_hh0)) \
        and not np.any(np.asarray(b_ih1)) and not np.any(np.asarray(b_hh1)) \
        and not np.any(np.asarray(proj_b)), "nonzero biases unsupported"

    # host: embedding gather, token order t*B + b
    x = emb[input_ids[:, :-1]]                       # [B, S, E]
    xT = np.ascontiguousarray(
        x.transpose(2, 1, 0).reshape(E, S * B))      # [E, TOK] col = t*B+b
    xT16 = xT.astype(np.float16)

    # host: h0 init (fp32, exact-class)
    h0 = initial_state @ enc_proj_W.T                # [B, H]
    h0T16 = np.ascontiguousarray(h0.T).astype(np.float16)   # [H, B]

    def gate_rows(c):
        base = np.arange(HS) + c * HS
        return np.concatenate([base, H + base, 3 * H + base, 2 * H + base])

    in_maps = []
    for c in range(NC):
        rows = gate_rows(c)
        wih0 = np.asarray(W_ih0, dtype=np.float32)[rows]      # [GS, E]
        whh0 = np.asarray(W_hh0, dtype=np.float32)[rows]      # [GS, H]
        wih1 = np.asarray(W_ih1, dtype=np.float32)[rows]      # [GS, H]
        whh1 = np.asarray(W_hh1, dtype=np.float32)[rows]      # [GS, H]
        w1 = np.concatenate([wih1, whh1], axis=1)             # [GS, 2H]
        in_maps.append({
            "xembT": xT16,
            "wih0T": np.ascontiguousarray(wih0.T).astype(np.float16),
            "whh0T": np.ascontiguousarray(whh0.T).astype(np.float16),
            "w1T": np.ascontiguousarray(w1.T).astype(np.float16),
            "h0T16": h0T16,
            "h0cT16": np.ascontiguousarray(h0T16[c * HS:(c + 1) * HS]),
            "c0": np.ascontiguousarray(h0[:, c * HS:(c + 1) * HS]),
            "projT": np.ascontiguousarray(proj_W[c * VS:(c + 1) * VS].T),
        })

    nc = _get_program()
    res = run_bass_kernel_spmd(nc, in_maps, core_ids=list(range(NC)))

    full = np.concatenate([res.results[c]["logits"] for c in range(NC)], axis=1)
    logits = np.ascontiguousarray(
        full.reshape(S, B, V).transpose(1, 0, 2))    # [B, S, V]
    preds = np.argmax(logits, axis=-1).astype(np.int32)
    return preds, logits


# revision 16
# speedup vs baseline: 1.0160x; 1.0160x over previous
"""Trainium2 Bass kernel for nn_DefinitionProbingLSTM.

Structure (8-core SPMD, one chip):
  - hidden units H=1024 sharded 8 ways (128 per core); each core computes its
    512 gate rows per LSTM layer per step (matmuls in fp16, batch on M=64).
  - one AllGather per step carries {h0n(t), h1n(t-1)} transposed slices, so
    layer 1 lags one gather behind layer 0 and only one collective sits on the
    serial critical chain.
  - output projection vocab-sharded (4000 vocab rows/core), fp32r matmuls,
    interleaved with the recurrence every two steps (token tiles M=128).
  - h0 = initial_state @ enc_proj_W.T computed on host in fp32 (tiny).
  - embedding gather, weight layout permutation/transposition and final argmax
    run on host inside kernel().
"""

import numpy as np
import ml_dtypes

import concourse.bass as bass
import concourse.mybir as mybir
import concourse.tile as tile
from concourse import bacc
from concourse.bass_utils import run_bass_kernel_spmd
from concourse.masks import make_identity

F16 = mybir.dt.float16
F32 = mybir.dt.float32
F32R = mybir.dt.float32r
AF = mybir.ActivationFunctionType

B, T, V, E, H, EH = 64, 64, 32000, 512, 1024, 2048
S = T - 1                    # 63 recurrence steps
NC = 8                       # cores
HS = H // NC                 # 128 hidden units per core
GS = 4 * HS                  # 512 gate rows per core per layer
VS = V // NC                 # 4000 vocab rows per core
TOK = S * B                  # 4032 tokens, order t*B + b
NTILE = (TOK + 127) // 128   # 32 token tiles (last one half: 64 rows)
NCHUNK = 8                   # vocab chunks of 500 per core
CW = VS // NCHUNK            # 500


def _build_program():
    nc = bacc.Bacc("TRN2", target_bir_lowering=False, debug=False,
                   num_devices=NC)

    d_xembT = nc.dram_tensor("xembT", [E, TOK], F16, kind="ExternalInput").ap()
    d_wih0T = nc.dram_tensor("wih0T", [E, GS], F16, kind="ExternalInput").ap()
    d_whh0T = nc.dram_tensor("whh0T", [H, GS], F16, kind="ExternalInput").ap()
    d_w1T = nc.dram_tensor("w1T", [2 * H, GS], F16, kind="ExternalInput").ap()
    d_h0T = nc.dram_tensor("h0T16", [H, B], F16, kind="ExternalInput").ap()
    d_h0cT = nc.dram_tensor("h0cT16", [128, B], F16, kind="ExternalInput").ap()
    d_c0 = nc.dram_tensor("c0", [B, HS], F32, kind="ExternalInput").ap()
    d_projT = nc.dram_tensor("projT", [H, VS], F32R, kind="ExternalInput").ap()
    d_logits = nc.dram_tensor("logits", [TOK, VS], F32, kind="ExternalOutput").ap()

    with tile.TileContext(nc) as tc:
        with tc.tile_pool(name="wpool", bufs=1) as wpool, \
             tc.tile_pool(name="big", bufs=1) as bigpool, \
             tc.tile_pool(name="xs", bufs=2) as xs, \
             tc.tile_pool(name="gath", bufs=4) as gath, \
             tc.tile_pool(name="work", bufs=2) as work, \
             tc.tile_pool(name="stage", bufs=4) as stpool, \
             tc.tile_pool(name="pl", bufs=2) as plpool, \
             tc.tile_pool(name="psA", bufs=2, space="PSUM") as psA, \
             tc.tile_pool(name="psB", bufs=1, space="PSUM") as psB, \
             tc.tile_pool(name="psT", bufs=2, space="PSUM") as psT, \
             tc.tile_pool(name="psP", bufs=3, space="PSUM") as psP, \
             tc.tile_pool(name="dr", bufs=1, space="DRAM") as dr, \
             tc.tile_pool(name="drag", bufs=3, space="DRAM") as drag:

            # ---------- resident weights ----------
            s_wih0 = wpool.tile([128, E // 128, GS], F16, tag="wih0")
            nc.sync.dma_start(s_wih0[:], d_wih0T.rearrange("(k p) g -> p k g", p=128))
            s_whh0 = wpool.tile([128, H // 128, GS], F16, tag="whh0")
            nc.sync.dma_start(s_whh0[:], d_whh0T.rearrange("(k p) g -> p k g", p=128))
            s_w1 = wpool.tile([128, 2 * H // 128, GS], F16, tag="w1")
            nc.sync.dma_start(s_w1[:], d_w1T.rearrange("(k p) g -> p k g", p=128))
            s_ident = wpool.tile([128, 128], F16, tag="ident")
            make_identity(nc, s_ident[:])

            # ---------- phase 0: initial states (host-computed) ----------
            c_l0 = work.tile([B, HS], F32, tag="c0", name="c_l0_init")
            nc.sync.dma_start(c_l0[:], d_c0)
            c_l1 = work.tile([B, HS], F32, tag="c1", name="c_l1_init")
            nc.sync.dma_start(c_l1[:], d_c0)
            h0c_T = work.tile([128, B], F16, tag="h0cT")
            nc.sync.dma_start(h0c_T[:], d_h0cT)
            # G_{-1}: even slots = h0T k-tiles (odd slots unused at t=0)
            g_prev = gath.tile([128, 2 * NC, B], F16, tag="g", name="g_init")
            nc.sync.dma_start(g_prev[:, 0::2, :],
                              d_h0T.rearrange("(k p) b -> p k b", p=128))

            # ---------- phase 1: x_ih precompute -> DRAM ----------
            d_xih = dr.tile([TOK, GS], F16, tag="xih")
            for p in range(NTILE):
                rows = min(128, TOK - p * 128)
                xt = xs.tile([128, E // 128, 128], F16, tag="xemb")
                nc.gpsimd.dma_start(
                    xt[:, :, :rows],
                    d_xembT[:, p * 128:p * 128 + rows].rearrange(
                        "(k p) m -> p k m", p=128))
                psx = psP.tile([128, GS], F32, tag="pp")
                for k in range(E // 128):
                    nc.tensor.matmul(psx[:rows, :], xt[:, k, :rows],
                                     s_wih0[:, k, :],
                                     start=(k == 0), stop=(k == E // 128 - 1))
                st16 = stpool.tile([128, GS], F16, tag="st16")
                nc.vector.tensor_copy(st16[:rows, :], psx[:rows, :])
                nc.scalar.dma_start(d_xih[p * 128:p * 128 + rows, :], st16[:rows, :])

            # projection weights resident (the one big slot)
            s_proj = bigpool.tile([128, H // 128, VS], F32R, tag="big",
                                  name="s_proj")
            for k in range(H // 128):
                nc.gpsimd.dma_start(
                    s_proj[:, k, :],
                    d_projT[k * 128:(k + 1) * 128, :])

            # ---------- phase 2: recurrence + interleaved projection ----------
            h1n_T_evac = None     # DVE-evacuated h1n(t-1).T [128, B] fp16

            plT_cur = [None]

            def project_pair(gA, gB, p, lo=0, hi=NCHUNK):
                """project token tile p (steps 2p, 2p+1), vocab chunks
                [lo, hi); gA/gB hold h1 of the two steps in odd slots."""
                rows = min(128, TOK - p * 128)
                if lo == 0:
                    plT = plpool.tile([128, H // 128, 128], F32R, tag="plT")
                    nc.vector.tensor_copy(plT[:, :, 0:B], gA[:, 1::2, :])
                    if gB is not None:
                        nc.vector.tensor_copy(plT[:, :, B:2 * B], gB[:, 1::2, :])
                    plT_cur[0] = plT
                plT = plT_cur[0]
                for n in range(lo, hi):
                    psp = psP.tile([128, CW], F32, tag="pp")
                    for k in range(H // 128):
                        nc.tensor.matmul(psp[:rows, :], plT[:, k, :rows],
                                         s_proj[:, k, n * CW:(n + 1) * CW],
                                         start=(k == 0), stop=(k == H // 128 - 1))
                    st = stpool.tile([128, GS], F32, tag="st")
                    if n % 2 == 0:
                        nc.vector.tensor_copy(st[:rows, :CW], psp[:rows, :])
                    else:
                        nc.scalar.activation(st[:rows, :CW], psp[:rows, :], AF.Copy)
                    nc.gpsimd.dma_start(
                        d_logits[p * 128:p * 128 + rows, n * CW:(n + 1) * CW],
                        st[:rows, :CW])

            g_hist = {-1: g_prev}

            def nonlin(ps_or_sb, c_prev, tag_sfx, xih_sb=None):
                """gate nonlinearity chain -> (h_new fp16 [B,HS], c_new f32)."""
                if xih_sb is not None:
                    gates = work.tile([B, GS], F32, tag="gates" + tag_sfx)
                    nc.vector.tensor_add(gates[:], ps_or_sb[:], xih_sb[:])
                else:
                    gates = ps_or_sb
                sfo = work.tile([B, 3 * HS], F32, tag="sfo" + tag_sfx)
                nc.scalar.activation(sfo[:], gates[:, 0:3 * HS], AF.Sigmoid)
                tg = work.tile([B, HS], F32, tag="tg" + tag_sfx, bufs=1)
                nc.scalar.activation(tg[:], gates[:, 3 * HS:GS], AF.Tanh)
                t1 = work.tile([B, HS], F32, tag="t1" + tag_sfx, bufs=1)
                nc.vector.tensor_mul(t1[:], sfo[:, HS:2 * HS], c_prev[:])
                t2 = work.tile([B, HS], F32, tag="t2" + tag_sfx, bufs=1)
                nc.vector.tensor_mul(t2[:], sfo[:, 0:HS], tg[:])
                c_new = work.tile([B, HS], F32, tag="c" + tag_sfx)
                nc.vector.tensor_add(c_new[:], t1[:], t2[:])
                tc = work.tile([B, HS], F32, tag="tc" + tag_sfx, bufs=1)
                nc.scalar.activation(tc[:], c_new[:], AF.Tanh)
                hn = work.tile([B, HS], F16, tag="hn" + tag_sfx)
                nc.vector.tensor_mul(hn[:], sfo[:, 2 * HS:3 * HS], tc[:])
                return hn, c_new

            # pipeline: body t emits mm0(t) and layer-1 of step t-1, then
            # projection work (fills the AG(t) wait), then AG(t) carrying
            # {h0n(t), h1n(t-1)}.
            ps1_prev = None       # mm1(t-2) PSUM, consumed by nonlin1 here
            for t in range(S + 2):
                agin_sb = work.tile([128, 2, B], F16, tag="agin")
                # ---- layer 1 matmul for step t-2 first (its chain binds)
                if t >= 2 and t - 2 < S:
                    ps1 = psB.tile([B, GS], F32, tag="g1")
                    for k in range(NC):
                        nc.tensor.matmul(ps1[:], g_hist[t - 1][:, 2 * k + 1, :],
                                         s_w1[:, NC + k, :],
                                         start=(k == 0), stop=False)
                    for k in range(NC):
                        nc.tensor.matmul(ps1[:], g_hist[t - 2][:, 2 * k, :],
                                         s_w1[:, k, :],
                                         start=False, stop=(k == NC - 1))
                    ps1_prev = ps1
                # ---- layer 0 matmul for step t (xih folded in via identity)
                if t < S:
                    xih_sb = xs.tile([B, GS], F16, tag="xih_sb")
                    nc.gpsimd.dma_start(xih_sb[:], d_xih[t * B:(t + 1) * B, :])
                    ps0 = psA.tile([B, GS], F32, tag="g0")
                    nc.tensor.matmul(ps0[:], s_ident[:B, :B], xih_sb[:],
                                     start=True, stop=False)
                    for k in range(NC):
                        nc.tensor.matmul(ps0[:], g_hist[t - 1][:, 2 * k, :],
                                         s_whh0[:, k, :],
                                         start=False, stop=(k == NC - 1))
                # ---- nonlinearities (ACT/DVE run parallel to PE)
                if t < S:
                    h0n16, c_l0 = nonlin(ps0, c_l0, "0")
                    pst0 = psT.tile([128, B], F16, tag="tp")
                    nc.tensor.transpose(pst0[:], h0n16[:], s_ident[:B, :B])
                    nc.vector.tensor_copy(agin_sb[:, 0, :], pst0[:])
                if t >= 2 and t - 2 < S:
                    h1n16, c_l1 = nonlin(ps1_prev, c_l1, "1")
                    pst1 = psT.tile([128, B], F16, tag="tp")
                    nc.tensor.transpose(pst1[:], h1n16[:], s_ident[:B, :B])
                    ev = work.tile([128, B], F16, tag="h1nT")
                    nc.vector.tensor_copy(ev[:], pst1[:])
                    h1n_T_evac = ev
                # slot1 = h1n(t-2).T ; slot0 dummy after layer0 ends
                if t >= 2:
                    nc.vector.tensor_copy(agin_sb[:, 1, :], h1n_T_evac[:])
                else:
                    nc.vector.tensor_copy(agin_sb[:, 1, :], h0c_T[:])
                if t >= S:
                    nc.vector.tensor_copy(agin_sb[:, 0, :], h1n_T_evac[:])
                # ---- AllGather t: {h0n(t), h1n(t-2)}
                ag_in = drag.tile([2 * 128, B], F16, tag="agin_d")
                nc.sync.dma_start(
                    ag_in.rearrange("(j p) b -> p j b", p=128), agin_sb[:])
                ag_out = drag.tile([2 * NC * 128, B], F16, tag="agout_d",
                                   addr_space="Shared")
                nc.gpsimd.collective_compute(
                    "AllGather", mybir.AluOpType.bypass,
                    replica_groups=[list(range(NC))],
                    ins=[ag_in.opt()], outs=[ag_out.opt()],
                )
                g_t = gath.tile([128, 2 * NC, B], F16, tag="g")
                # odds (h1, feeds the binding mm1 chain) first, then evens
                nc.sync.dma_start(
                    g_t[:, 1::2, :],
                    ag_out.rearrange("(s p) b -> p s b", p=128)[:, 1::2, :])
                nc.sync.dma_start(
                    g_t[:, 0::2, :],
                    ag_out.rearrange("(s p) b -> p s b", p=128)[:, 0::2, :])
                g_hist[t] = g_t

                # ---- projection of pair p: h1(x) lives in G_{x+2}.odd
                # pair p = steps (2p, 2p+1) -> needs G_{2p+2}, G_{2p+3};
                # chunks split 4+4 across this body and the next
                if t >= 4 and t % 2 == 0:
                    project_pair(g_hist[t - 2], g_hist[t - 1], (t - 4) // 2,
                                 0, NCHUNK // 2)
                    del g_hist[t - 3]
                if t >= 5 and t % 2 == 1:
                    project_pair(None, None, (t - 5) // 2, NCHUNK // 2, NCHUNK)

            # tail: last pairs. h1(x) in G_{x+2}: pair 30 = steps 60,61
            # -> G_62, G_63 ; half tile 31 = step 62 -> G_64.
            project_pair(g_hist[S - 1], g_hist[S], NTILE - 2, NCHUNK // 2, NCHUNK)
            project_pair(g_hist[S + 1], None, NTILE - 1)

    nc.compile()
    return nc


_PROGRAM = None


def _get_program():
    global _PROGRAM
    if _PROGRAM is None:
        _PROGRAM = _build_program()
    return _PROGRAM


def kernel(input_ids, initial_state, emb, enc_proj_W, W_ih0, W_hh0, b_ih0,
           b_hh0, W_ih1, W_hh1, b_ih1, b_hh1, proj_W, proj_b):
    input_ids = np.asarray(input_ids)
    initial_state = np.asarray(initial_state, dtype=np.float32)
    emb = np.asarray(emb, dtype=np.float32)
    enc_proj_W = np.asarray(enc_proj_W, dtype=np.float32)
    proj_W = np.asarray(proj_W, dtype=np.float32)
    assert not np.any(np.asarray(b_ih0)) and not np.any(np.asarray(b_hh0)) \
        and not np.any(np.asarray(b_ih1)) and not np.any(np.asarray(b_hh1)) \
        and not np.any(np.asarray(proj_b)), "nonzero biases unsupported"

    # host: embedding gather, token order t*B + b
    x = emb[input_ids[:, :-1]]                       # [B, S, E]
    xT = np.ascontiguousarray(
        x.transpose(2, 1, 0).reshape(E, S * B))      # [E, TOK] col = t*B+b
    xT16 = xT.astype(np.float16)

    # host: h0 init (fp32, exact-class)
    h0 = initial_state @ enc_proj_W.T                # [B, H]
    h0T16 = np.ascontiguousarray(h0.T).astype(np.float16)   # [H, B]

    def gate_rows(c):
        base = np.arange(HS) + c * HS
        return np.concatenate([base, H + base, 3 * H + base, 2 * H + base])

    in_maps = []
    for c in range(NC):
        rows = gate_rows(c)
        wih0 = np.asarray(W_ih0, dtype=np.float32)[rows]      # [GS, E]
        whh0 = np.asarray(W_hh0, dtype=np.float32)[rows]      # [GS, H]
        wih1 = np.asarray(W_ih1, dtype=np.float32)[rows]      # [GS, H]
        whh1 = np.asarray(W_hh1, dtype=np.float32)[rows]      # [GS, H]
        w1 = np.concatenate([wih1, whh1], axis=1)             # [GS, 2H]
        in_maps.append({
            "xembT": xT16,
            "wih0T": np.ascontiguousarray(wih0.T).astype(np.float16),
            "whh0T": np.ascontiguousarray(whh0.T).astype(np.float16),
            "w1T": np.ascontiguousarray(w1.T).astype(np.float16),
            "h0T16": h0T16,
            "h0cT16": np.ascontiguousarray(h0T16[c * HS:(c + 1) * HS]),
            "c0": np.ascontiguousarray(h0[:, c * HS:(c + 1) * HS]),
            "projT": np.ascontiguousarray(proj_W[c * VS:(c + 1) * VS].T),
        })

    nc = _get_program()
    res = run_bass_kernel_spmd(nc, in_maps, core_ids=list(range(NC)))

    full = np.concatenate([res.results[c]["logits"] for c in range(NC)], axis=1)
    logits = np.ascontiguousarray(
        full.reshape(S, B, V).transpose(1, 0, 2))    # [B, S, V]
    preds = np.argmax(logits, axis=-1).astype(np.int32)
    return preds, logits


# revision 19
# speedup vs baseline: 1.0852x; 1.0681x over previous
"""Trainium2 Bass kernel for nn_DefinitionProbingLSTM.

Structure (8-core SPMD, one chip):
  - hidden units H=1024 sharded 8 ways (128 per core); each core computes its
    512 gate rows per LSTM layer per step (matmuls in fp16, batch on M=64).
  - one AllGather per step carries {h0n(t), h1n(t-1)} transposed slices, so
    layer 1 lags one gather behind layer 0 and only one collective sits on the
    serial critical chain.
  - output projection vocab-sharded (4000 vocab rows/core), fp32r matmuls,
    interleaved with the recurrence every two steps (token tiles M=128).
  - h0 = initial_state @ enc_proj_W.T computed on host in fp32 (tiny).
  - embedding gather, weight layout permutation/transposition and final argmax
    run on host inside kernel().
"""

import numpy as np
import ml_dtypes

import concourse.bass as bass
import concourse.mybir as mybir
import concourse.tile as tile
from concourse import bacc
from concourse.bass_utils import run_bass_kernel_spmd
from concourse.masks import make_identity

F16 = mybir.dt.float16
F32 = mybir.dt.float32
F32R = mybir.dt.float32r
AF = mybir.ActivationFunctionType

B, T, V, E, H, EH = 64, 64, 32000, 512, 1024, 2048
S = T - 1                    # 63 recurrence steps
NC = 8                       # cores
HS = H // NC                 # 128 hidden units per core
GS = 4 * HS                  # 512 gate rows per core per layer
VS = V // NC                 # 4000 vocab rows per core
TOK = S * B                  # 4032 tokens, order t*B + b
NTILE = (TOK + 127) // 128   # 32 token tiles (last one half: 64 rows)
NCHUNK = 8                   # vocab chunks of 500 per core
CW = VS // NCHUNK            # 500


def _build_program():
    nc = bacc.Bacc("TRN2", target_bir_lowering=False, debug=False,
                   num_devices=NC)

    d_xembT = nc.dram_tensor("xembT", [E, TOK], F16, kind="ExternalInput").ap()
    d_wih0T = nc.dram_tensor("wih0T", [E, GS], F16, kind="ExternalInput").ap()
    d_whh0T = nc.dram_tensor("whh0T", [H, GS], F16, kind="ExternalInput").ap()
    d_w1T = nc.dram_tensor("w1T", [2 * H, GS], F16, kind="ExternalInput").ap()
    d_h0T = nc.dram_tensor("h0T16", [H, B], F16, kind="ExternalInput").ap()
    d_h0cT = nc.dram_tensor("h0cT16", [128, B], F16, kind="ExternalInput").ap()
    d_c0 = nc.dram_tensor("c0", [B, HS], F32, kind="ExternalInput").ap()
    d_projT = nc.dram_tensor("projT", [H, VS], F32R, kind="ExternalInput").ap()
    d_logits = nc.dram_tensor("logits", [TOK, VS], F32, kind="ExternalOutput").ap()

    with tile.TileContext(nc) as tc:
        with tc.tile_pool(name="wpool", bufs=1) as wpool, \
             tc.tile_pool(name="big", bufs=1) as bigpool, \
             tc.tile_pool(name="xs", bufs=2) as xs, \
             tc.tile_pool(name="gath", bufs=4) as gath, \
             tc.tile_pool(name="work", bufs=2) as work, \
             tc.tile_pool(name="stage", bufs=4) as stpool, \
             tc.tile_pool(name="pl", bufs=2) as plpool, \
             tc.tile_pool(name="psA", bufs=2, space="PSUM") as psA, \
             tc.tile_pool(name="psB", bufs=2, space="PSUM") as psB, \
             tc.tile_pool(name="psT", bufs=2, space="PSUM") as psT, \
             tc.tile_pool(name="psP", bufs=2, space="PSUM") as psP, \
             tc.tile_pool(name="dr", bufs=1, space="DRAM") as dr, \
             tc.tile_pool(name="drag", bufs=3, space="DRAM") as drag:

            # ---------- resident weights ----------
            s_wih0 = wpool.tile([128, E // 128, GS], F16, tag="wih0")
            nc.sync.dma_start(s_wih0[:], d_wih0T.rearrange("(k p) g -> p k g", p=128))
            s_whh0 = wpool.tile([128, H // 128, GS], F16, tag="whh0")
            nc.sync.dma_start(s_whh0[:], d_whh0T.rearrange("(k p) g -> p k g", p=128))
            s_w1 = wpool.tile([128, 2 * H // 128, GS], F16, tag="w1")
            nc.sync.dma_start(s_w1[:], d_w1T.rearrange("(k p) g -> p k g", p=128))
            s_ident = wpool.tile([128, 128], F16, tag="ident")
            make_identity(nc, s_ident[:])

            # ---------- phase 0: initial states (host-computed) ----------
            c_l0 = work.tile([B, HS], F32, tag="c0", name="c_l0_init")
            nc.sync.dma_start(c_l0[:], d_c0)
            c_l1 = work.tile([B, HS], F32, tag="c1", name="c_l1_init")
            nc.sync.dma_start(c_l1[:], d_c0)
            h0c_T = work.tile([128, B], F16, tag="h0cT")
            nc.sync.dma_start(h0c_T[:], d_h0cT)
            # G_{-1}: even slots = h0T k-tiles (odd slots unused at t=0)
            g_prev = gath.tile([128, 2 * NC, B], F16, tag="g", name="g_init")
            nc.sync.dma_start(g_prev[:, 0::2, :],
                              d_h0T.rearrange("(k p) b -> p k b", p=128))

            # ---------- phase 1: x_ih precompute -> DRAM ----------
            d_xih = dr.tile([TOK, GS], F16, tag="xih")
            for p in range(NTILE):
                rows = min(128, TOK - p * 128)
                xt = xs.tile([128, E // 128, 128], F16, tag="xemb")
                nc.gpsimd.dma_start(
                    xt[:, :, :rows],
                    d_xembT[:, p * 128:p * 128 + rows].rearrange(
                        "(k p) m -> p k m", p=128))
                psx = psP.tile([128, GS], F32, tag="pp")
                for k in range(E // 128):
                    nc.tensor.matmul(psx[:rows, :], xt[:, k, :rows],
                                     s_wih0[:, k, :],
                                     start=(k == 0), stop=(k == E // 128 - 1))
                st16 = stpool.tile([128, GS], F16, tag="st16")
                nc.vector.tensor_copy(st16[:rows, :], psx[:rows, :])
                nc.scalar.dma_start(d_xih[p * 128:p * 128 + rows, :], st16[:rows, :])

            # projection weights resident (the one big slot)
            s_proj = bigpool.tile([128, H // 128, VS], F32R, tag="big",
                                  name="s_proj")
            for k in range(H // 128):
                nc.gpsimd.dma_start(
                    s_proj[:, k, :],
                    d_projT[k * 128:(k + 1) * 128, :])

            # ---------- phase 2: recurrence + interleaved projection ----------
            h1n_T_evac = None     # DVE-evacuated h1n(t-1).T [128, B] fp16

            plT_cur = [None]

            def project_pair(gA, gB, p, lo=0, hi=NCHUNK):
                """project token tile p (steps 2p, 2p+1), vocab chunks
                [lo, hi); gA/gB hold h1 of the two steps in odd slots."""
                rows = min(128, TOK - p * 128)
                if lo == 0:
                    plT = plpool.tile([128, H // 128, 128], F32R, tag="plT")
                    nc.vector.tensor_copy(plT[:, :, 0:B], gA[:, 1::2, :])
                    if gB is not None:
                        nc.vector.tensor_copy(plT[:, :, B:2 * B], gB[:, 1::2, :])
                    plT_cur[0] = plT
                plT = plT_cur[0]
                for n in range(lo, hi):
                    psp = psP.tile([128, CW], F32, tag="pp")
                    for k in range(H // 128):
                        nc.tensor.matmul(psp[:rows, :], plT[:, k, :rows],
                                         s_proj[:, k, n * CW:(n + 1) * CW],
                                         start=(k == 0), stop=(k == H // 128 - 1))
                    st = stpool.tile([128, GS], F32, tag="st")
                    if n % 2 == 0:
                        nc.vector.tensor_copy(st[:rows, :CW], psp[:rows, :])
                    else:
                        nc.scalar.activation(st[:rows, :CW], psp[:rows, :], AF.Copy)
                    nc.gpsimd.dma_start(
                        d_logits[p * 128:p * 128 + rows, n * CW:(n + 1) * CW],
                        st[:rows, :CW])

            g_hist = {-1: g_prev}

            def nonlin(ps_or_sb, c_prev, tag_sfx, xih_sb=None):
                """gate nonlinearity chain -> (h_new fp16 [B,HS], c_new f32)."""
                if xih_sb is not None:
                    gates = work.tile([B, GS], F32, tag="gates" + tag_sfx)
                    nc.vector.tensor_add(gates[:], ps_or_sb[:], xih_sb[:])
                else:
                    gates = ps_or_sb
                sfo = work.tile([B, 3 * HS], F32, tag="sfo" + tag_sfx)
                nc.scalar.activation(sfo[:], gates[:, 0:3 * HS], AF.Sigmoid)
                tg = work.tile([B, HS], F32, tag="tg" + tag_sfx, bufs=1)
                nc.scalar.activation(tg[:], gates[:, 3 * HS:GS], AF.Tanh)
                t1 = work.tile([B, HS], F32, tag="t1" + tag_sfx, bufs=1)
                nc.vector.tensor_mul(t1[:], sfo[:, HS:2 * HS], c_prev[:])
                t2 = work.tile([B, HS], F32, tag="t2" + tag_sfx, bufs=1)
                nc.vector.tensor_mul(t2[:], sfo[:, 0:HS], tg[:])
                c_new = work.tile([B, HS], F32, tag="c" + tag_sfx)
                nc.vector.tensor_add(c_new[:], t1[:], t2[:])
                tc = work.tile([B, HS], F32, tag="tc" + tag_sfx, bufs=1)
                nc.scalar.activation(tc[:], c_new[:], AF.Tanh)
                hn = work.tile([B, HS], F16, tag="hn" + tag_sfx)
                nc.vector.tensor_mul(hn[:], sfo[:, 2 * HS:3 * HS], tc[:])
                return hn, c_new

            # pipeline: body t emits mm0(t) and layer-1 of step t-1, then
            # projection work (fills the AG(t) wait), then AG(t) carrying
            # {h0n(t), h1n(t-1)}.
            ps1_cur = None        # mm1(t-2) PSUM: IH1 half emitted last body
            ps1_next = None
            pst1 = None
            for t in range(S + 2):
                # ---- finish layer-1 matmul for step t-2 (HH1 half, fresh odds)
                if t >= 2 and t - 2 < S:
                    for k in range(NC):
                        nc.tensor.matmul(ps1_cur[:],
                                         g_hist[t - 1][:, 2 * k + 1, :],
                                         s_w1[:, NC + k, :],
                                         start=False, stop=(k == NC - 1))
                # ---- layer 0 matmul for step t (xih folded in via identity)
                if t < S:
                    xih_sb = xs.tile([B, GS], F16, tag="xih_sb")
                    nc.gpsimd.dma_start(xih_sb[:], d_xih[t * B:(t + 1) * B, :])
                    ps0 = psA.tile([B, GS], F32, tag="g0")
                    nc.tensor.matmul(ps0[:], s_ident[:B, :B], xih_sb[:],
                                     start=True, stop=False)
                    for k in range(NC):
                        nc.tensor.matmul(ps0[:], g_hist[t - 1][:, 2 * k, :],
                                         s_whh0[:, k, :],
                                         start=False, stop=(k == NC - 1))
                # ---- nonlinearities + transposes (feed AG directly from PSUM)
                ag_in = drag.tile([2 * 128, B], F16, tag="agin_d")
                if t < S:
                    h0n16, c_l0 = nonlin(ps0, c_l0, "0")
                    pst0 = psT.tile([128, B], F16, tag="tp", bufs=1)
                    nc.tensor.transpose(pst0[:], h0n16[:], s_ident[:B, :B])
                    a0 = work.tile([128, B], F16, tag="a0")
                    nc.vector.tensor_copy(a0[:], pst0[:])
                    nc.sync.dma_start(ag_in[0:128, :], a0[:])
                if t >= 2 and t - 2 < S:
                    h1n16, c_l1 = nonlin(ps1_cur, c_l1, "1")
                    pst1 = psT.tile([128, B], F16, tag="tp1", bufs=1)
                    nc.tensor.transpose(pst1[:], h1n16[:], s_ident[:B, :B])
                    a1 = work.tile([128, B], F16, tag="a1")
                    nc.vector.tensor_copy(a1[:], pst1[:])
                    a1_cur = a1
                if t >= 2:
                    nc.sync.dma_start(ag_in[128:256, :], a1_cur[:])
                else:
                    nc.sync.dma_start(ag_in[128:256, :], h0c_T[:])
                if t >= S:
                    nc.sync.dma_start(ag_in[0:128, :], a1_cur[:])
                # ---- pre-emit IH1 half of layer-1 for step t-1 (off-chain)
                if t >= 1 and t - 1 < S:
                    ps1_next = psB.tile([B, GS], F32, tag="g1")
                    for k in range(NC):
                        nc.tensor.matmul(ps1_next[:], g_hist[t - 1][:, 2 * k, :],
                                         s_w1[:, k, :],
                                         start=(k == 0), stop=False)
                ps1_cur = ps1_next
                # ---- AllGather t: {h0n(t), h1n(t-2)}
                ag_out = drag.tile([2 * NC * 128, B], F16, tag="agout_d",
                                   addr_space="Shared")
                nc.gpsimd.collective_compute(
                    "AllGather", mybir.AluOpType.bypass,
                    replica_groups=[list(range(NC))],
                    ins=[ag_in.opt()], outs=[ag_out.opt()],
                )
                g_t = gath.tile([128, 2 * NC, B], F16, tag="g")
                # odds (h1, feeds the binding mm1 chain) first, then evens
                nc.sync.dma_start(
                    g_t[:, 1::2, :],
                    ag_out.rearrange("(s p) b -> p s b", p=128)[:, 1::2, :])
                nc.sync.dma_start(
                    g_t[:, 0::2, :],
                    ag_out.rearrange("(s p) b -> p s b", p=128)[:, 0::2, :])
                g_hist[t] = g_t

                # ---- projection of pair p: h1(x) lives in G_{x+2}.odd
                # pair p = steps (2p, 2p+1) -> needs G_{2p+2}, G_{2p+3};
                # chunks split 4+4 across this body and the next
                if t >= 4 and t % 2 == 0:
                    project_pair(g_hist[t - 2], g_hist[t - 1], (t - 4) // 2,
                                 0, NCHUNK // 2)
                    del g_hist[t - 3]
                if t >= 5 and t % 2 == 1:
                    project_pair(None, None, (t - 5) // 2, NCHUNK // 2, NCHUNK)

            # tail: last pairs. h1(x) in G_{x+2}: pair 30 = steps 60,61
            # -> G_62, G_63 ; half tile 31 = step 62 -> G_64.
            project_pair(g_hist[S - 1], g_hist[S], NTILE - 2, NCHUNK // 2, NCHUNK)
            project_pair(g_hist[S + 1], None, NTILE - 1)

    nc.compile()
    return nc


_PROGRAM = None


def _get_program():
    global _PROGRAM
    if _PROGRAM is None:
        _PROGRAM = _build_program()
    return _PROGRAM


def kernel(input_ids, initial_state, emb, enc_proj_W, W_ih0, W_hh0, b_ih0,
           b_hh0, W_ih1, W_hh1, b_ih1, b_hh1, proj_W, proj_b):
    input_ids = np.asarray(input_ids)
    initial_state = np.asarray(initial_state, dtype=np.float32)
    emb = np.asarray(emb, dtype=np.float32)
    enc_proj_W = np.asarray(enc_proj_W, dtype=np.float32)
    proj_W = np.asarray(proj_W, dtype=np.float32)
    assert not np.any(np.asarray(b_ih0)) and not np.any(np.asarray(b_hh0)) \
        and not np.any(np.asarray(b_ih1)) and not np.any(np.asarray(b_hh1)) \
        and not np.any(np.asarray(proj_b)), "nonzero biases unsupported"

    # host: embedding gather, token order t*B + b
    x = emb[input_ids[:, :-1]]                       # [B, S, E]
    xT = np.ascontiguousarray(
        x.transpose(2, 1, 0).reshape(E, S * B))      # [E, TOK] col = t*B+b
    xT16 = xT.astype(np.float16)

    # host: h0 init (fp32, exact-class)
    h0 = initial_state @ enc_proj_W.T                # [B, H]
    h0T16 = np.ascontiguousarray(h0.T).astype(np.float16)   # [H, B]

    def gate_rows(c):
        base = np.arange(HS) + c * HS
        return np.concatenate([base, H + base, 3 * H + base, 2 * H + base])

    in_maps = []
    for c in range(NC):
        rows = gate_rows(c)
        wih0 = np.asarray(W_ih0, dtype=np.float32)[rows]      # [GS, E]
        whh0 = np.asarray(W_hh0, dtype=np.float32)[rows]      # [GS, H]
        wih1 = np.asarray(W_ih1, dtype=np.float32)[rows]      # [GS, H]
        whh1 = np.asarray(W_hh1, dtype=np.float32)[rows]      # [GS, H]
        w1 = np.concatenate([wih1, whh1], axis=1)             # [GS, 2H]
        in_maps.append({
            "xembT": xT16,
            "wih0T": np.ascontiguousarray(wih0.T).astype(np.float16),
            "whh0T": np.ascontiguousarray(whh0.T).astype(np.float16),
            "w1T": np.ascontiguousarray(w1.T).astype(np.float16),
            "h0T16": h0T16,
            "h0cT16": np.ascontiguousarray(h0T16[c * HS:(c + 1) * HS]),
            "c0": np.ascontiguousarray(h0[:, c * HS:(c + 1) * HS]),
            "projT": np.ascontiguousarray(proj_W[c * VS:(c + 1) * VS].T),
        })

    nc = _get_program()
    res = run_bass_kernel_spmd(nc, in_maps, core_ids=list(range(NC)))

    full = np.concatenate([res.results[c]["logits"] for c in range(NC)], axis=1)
    logits = np.ascontiguousarray(
        full.reshape(S, B, V).transpose(1, 0, 2))    # [B, S, V]
    preds = np.argmax(logits, axis=-1).astype(np.int32)
    return preds, logits
